# revision 28
# baseline (speedup 1.0000x reference)
"""Trainium2 Bass kernel for nn_Block_9345848836513.

Per-core pipeline (8 cores = 4 batches x 2 channel-halves, 16 ch each):
  1. channel mix in fp16 on PE: lhsT = x-chunk [128 rows = 4 consecutive
     128-tau windows x 32 ch, 128 taus], rhs = block-diag 4x mixer
     [128, 64] -> psum [tau, (window, ch)]; full 128-row contraction,
     one psum tag.  x arrives from HBM as fp16 (halves the phase-A DMA,
     its critical path).  Copies scatter psum into BigX laid out
     [p, ch, j-parity, frame] (fp16) so later folds read contiguous rows.
  2. forward rfft of 512-sample frames (hop 256) via radix-4-folded real
     DFT: DVE/Pool folds produce C1/C2 (even-bin sources) and B0/B1
     (odd-bin halves); 6 bf16 matmuls per channel give all 512 real DOFs.
  3. per-frame recurrence out_i = (spec_i + out_{i-1}) * transfer via
     tensor_tensor_scan along the frame axis (fp32 state, bf16 out)
  4. inverse rfft with Hann folded into the bf16 matrix; overlap-add
     folded into PSUM accumulation (second matmul group reads with a
     one-column shift); tanh straight from PSUM.

Single-shot latency optimization: the whole transform chain is CHUNKED
along the frame axis (nchunk chunks of F/nchunk frames).  The scan
chains across chunks via its `initial` operand (the previous chunk's
last output column), so phase B for chunk c starts as soon as the x
tiles covering its frames (+1 boundary slot) have arrived - phase A DMA
and mix overlap phase B of earlier chunks instead of serializing in
front of the whole transform.  b0/b1 folds ride the Pool engine and
output DMAs ride the idle SP HWDGE queue to keep DVE under the PE
roofline.  u spectra live in 64 full-span [128,513] tiles so chunk
boundaries need no copies.
"""

import numpy as np

import concourse.bass as bass
import concourse.mybir as mybir
import concourse.tile as tile
from concourse import bacc
from concourse.bass_utils import run_bass_kernel_spmd

WINDOW = 512
HOP = 256
NCOEF = 257
NDOF = 512
B, C, T = 4, 32, 131072
F = T // HOP          # 512 frames
CPC = 16              # channels per core
NCORES = 8
JCOLS = T // 128      # 1024 output columns per channel
FPAD = F + 1          # 513 frame slots per (ch, parity); last is zero pad
FP32 = mybir.dt.float32
FP32R = mybir.dt.float32r
FP16 = mybir.dt.float16
BF16 = mybir.dt.bfloat16
U16 = mybir.dt.uint16


def _build_dft_matrices():
    w = np.arange(WINDOW)
    k = np.arange(NCOEF)
    ang = 2.0 * np.pi * np.outer(w, k) / WINDOW
    cos, sin = np.cos(ang), np.sin(ang)
    fmat = np.zeros((WINDOW, NDOF), np.float64)
    fmat[:, :NCOEF] = cos
    fmat[:, NCOEF:] = -sin[:, 1:256]
    hann = 0.5 - 0.5 * np.cos(2.0 * np.pi * w / WINDOW)
    g = np.zeros((NDOF, WINDOW), np.float64)
    g[0, :] = 1.0
    g[256, :] = cos[:, 256]
    for kk in range(1, 256):
        g[kk, :] = 2.0 * cos[:, kk]
        g[256 + kk, :] = -2.0 * sin[:, kk]
    g *= hann[None, :] / WINDOW

    # dof indexing in the plain layout: Re k -> k (0..256), Im k -> 256+k
    def dofs_re(ks):
        return list(ks)

    def dofs_im(ks):
        return [256 + kk for kk in ks if 1 <= kk <= 255]

    # Chunks of 128 dofs in bin-class order (radix-4): classes k mod 4 =
    # 0/2/1/3 with fold sources C1 = x0+x1+x2+x3, C2 = x0-x1+x2-x3 (via
    # e-halves eh0 = x0+x2, eh1 = x1+x3), B0 = x0-x2, B1 = x1-x3; for
    # k=0 mod 4: X[k] = sum_n C1[n] W^nk; k=2 mod 4: C2; odd k:
    # X[k] = sum_n B0[n] W^nk + B1[n] W^(n+128)k.
    chunk_k = [
        list(range(0, 257, 4)),
        list(range(2, 256, 4)),
        list(range(1, 256, 4)),
        list(range(3, 256, 4)),
    ]
    newperm = []
    for ks in chunk_k:
        newperm += dofs_re(ks) + dofs_im(ks)
    newperm = np.asarray(newperm)
    assert newperm.size == NDOF and np.unique(newperm).size == NDOF

    p0, p1, p2, p3 = (newperm[128 * i:128 * (i + 1)] for i in range(4))
    # 6 lhsT blocks: (C1->c0, C2->c1, B0->c2, B1->c2, B0->c3, B1->c3)
    f_blocks = np.stack([
        fmat[0:128, p0],
        fmat[0:128, p1],
        fmat[0:128, p2],
        fmat[128:256, p2],
        fmat[0:128, p3],
        fmat[128:256, p3],
    ], axis=1)                                    # [128, 6, 128]
    g2 = g[newperm, :]                                # [512, 512]
    g_l = g2.reshape(4, 128, 4, 128).transpose(1, 0, 2, 3)
    bf16 = mybir.dt.np(BF16)
    return (np.ascontiguousarray(f_blocks.astype(bf16)),
            np.ascontiguousarray(g_l.astype(bf16)), newperm)


def _build_program(detect_races=True, reps=1, chunks=(192, 320), pool_folds=2,
                   out_eng="scalar", psp_bufs=3, pwp_bufs=3, xa_bufs=3,
                   eo_bufs=3, rp_bufs=3, ilv=3, scan_pool=0, lag=1,
                   copy_split=("scalar", "scalar", "scalar", "scalar"),
                   serialize=False, tail_split=2):
    nc = bacc.Bacc("TRN2", target_bir_lowering=False, num_devices=NCORES,
                   detect_race_conditions=detect_races)
    # xq[32*b + c, G*128 + p] = x[c, G*512 + b*128 + p]  (fp16, host layout)
    xq = nc.dram_tensor("xq", [128, T // 4], FP16, kind="ExternalInput")
    # mix4[32*b + c, 16*b + d] = mixer[c, d0+d]; zero elsewhere (fp16)
    mix4 = nc.dram_tensor("mix4", [128, 4 * CPC], FP16, kind="ExternalInput")
    fmat = nc.dram_tensor("fmat", [128, 6, 128], BF16, kind="ExternalInput")
    gmat = nc.dram_tensor("gmat", [128, 4, 4, 128], BF16, kind="ExternalInput")
    trt = nc.dram_tensor("trt", [128, CPC * 4], FP32, kind="ExternalInput")
    gainv = nc.dram_tensor("gainv", [1, CPC], FP32, kind="ExternalInput")
    out_d = nc.dram_tensor("out", [CPC, 128, JCOLS], FP32, kind="ExternalOutput")

    ADD, MUL = mybir.AluOpType.add, mybir.AluOpType.mult
    SUB = mybir.AluOpType.subtract

    chunks = tuple(chunks)
    nchunk = len(chunks)
    assert sum(chunks) == F and all(c > 0 for c in chunks)
    cum = [0]
    for c in chunks:
        cum.append(cum[-1] + c)

    with tile.TileContext(nc) as tc:
        with (
            tc.tile_pool(name="singles", bufs=1) as singles,
            tc.tile_pool(name="xa", bufs=xa_bufs) as xa,
            tc.tile_pool(name="pmix", bufs=2, space="PSUM") as pmix,
            tc.tile_pool(name="eo", bufs=eo_bufs) as eop,
            tc.tile_pool(name="rp", bufs=rp_bufs) as rp,
            tc.tile_pool(name="psp", bufs=psp_bufs, space="PSUM") as psp,
            tc.tile_pool(name="pwp", bufs=pwp_bufs, space="PSUM") as pwp,
        ):
            fsb = singles.tile([128, 6, 128], BF16)
            gsb = singles.tile([128, 4, 4, 128], BF16)
            mix_sb = singles.tile([128, 4 * CPC], FP16)
            trsb = singles.tile([128, CPC * 4], FP32)
            gain_sb = singles.tile([128, CPC], FP32)
            # bigx[p, d, h, f] = y[d, t = 256f + 128h + p]  (j = 2f+h)
            bigxs = [singles.tile([128, CPC, 2, FPAD], FP16, name=f"bigx{r}")
                     for r in range(min(reps, 2))]
            # one full-span u tile per (channel, dof-chunk): the scan writes
            # chunk frames at col offset 1, so col f0 is the previous
            # chunk's last output (scan initial + shifted OLA read) and
            # col 0 is the zero pad for frame -1.
            u_all = [singles.tile([128, FPAD], BF16, name=f"uall{i}")
                     for i in range(CPC * 4)]
            for ut in u_all:
                nc.vector.memset(ut[:, 0:1].bitcast(U16), 0)
            # preload the tanh activation table during the DMA-idle fill
            # window (the implicit load costs 1.28us and would otherwise
            # block phase-A psum copies at the first real tanh)
            warm = singles.tile([128, 1], FP32)
            nc.vector.memset(warm[:], 0)
            nc.scalar.activation(warm[:], warm[:],
                                 mybir.ActivationFunctionType.Tanh)
            # params ride the SWDGE queue so the first phase-A x-load
            # (HWDGE) isn't queued behind the parameter DMAs
            nc.sync.dma_start(out=mix_sb[:], in_=mix4[:])
            nc.gpsimd.dma_start(out=fsb[:], in_=fmat[:])
            nc.gpsimd.dma_start(out=gsb[:], in_=gmat[:])
            nc.gpsimd.dma_start(out=trsb[:], in_=trt[:])
            nc.gpsimd.dma_start(out=gain_sb[:], in_=gainv[:].to_broadcast((128, CPC)))
            for bx in bigxs:
                nc.vector.memset(bx[:, :, :, F:FPAD].bitcast(U16), 0)

            def bigx_copy(eng, dst, src):
                if eng == "scalar":
                    nc.scalar.copy(dst, src)
                elif eng == "vector":
                    nc.vector.tensor_scalar(dst, src, 0.0, None, op0=ADD)
                else:
                    nc.gpsimd.tensor_scalar(dst, src, 0.0, None, op0=ADD)

            def emit_a_tile(bigx, i):
                # ---- Phase A: transposed mix into BigX (fp16) ----
                # xt tile i holds groups G in [32i, 32(i+1)); matmul for
                # group G: lhsT = xt[:, local window] -> psum cols
                # [64*G' + 16*b + d] with j = 4G + b, h = b%2,
                # f = 2*(8s + G') + b//2  (s = psum slab index 4i+g).
                xt = xa.tile([128, 4096], FP16, tag="xt")
                nc.sync.dma_start(
                    out=xt[:], in_=xq[:, 4096 * i:4096 * (i + 1)])
                for g in range(4):
                    ps = pmix.tile([128, 512], FP32, tag="pmix")
                    for gp in range(8):
                        nc.tensor.matmul(
                            ps[:, 64 * gp:64 * (gp + 1)],
                            lhsT=xt[:, 1024 * g + 128 * gp:
                                    1024 * g + 128 * (gp + 1)],
                            rhs=mix_sb[:],
                        )
                    # psum col = G'*64 + b*16 + d, with b = 2*bf + h
                    # and frame offset fr = 2*G' + bf, i.e.
                    # col = 32*fr + 16*h + d -> "(fr h d)" grouping.
                    f0 = 16 * (4 * i + g)
                    bigx_copy(
                        copy_split[g],
                        bigx[:, :, :, f0:f0 + 16],
                        ps[:].rearrange("p (fr h d) -> p d h fr",
                                        fr=16, h=2),
                    )

            # global i-tile plan across reps; tiles are emitted lazily so
            # phase A interleaves with phase B of earlier chunks
            tile_plan = [(r, i) for r in range(reps) for i in range(8)]
            state = {"ptr": 0}

            def bigx_for(r):
                return bigxs[r % len(bigxs)]

            def emit_until(goal):
                # emit pending i-tiles up to global index `goal` (inclusive)
                while state["ptr"] <= min(goal, len(tile_plan) - 1):
                    r, i = tile_plan[state["ptr"]]
                    emit_a_tile(bigx_for(r), i)
                    state["ptr"] += 1

            def chunk_need(r, c):
                # last global tile index chunk (r, c) reads: tiles through
                # the one whose first copy group covers the fold boundary
                # frame slot, except for the rep's last chunk whose
                # boundary is the zero pad.
                last_i = cum[c + 1] // 64 if c < nchunk - 1 else 7
                return 8 * r + last_i

            pending = []

            def emit_inverse(f0, FC, d):
                # ---- inverse DFT with overlap-add in PSUM ----
                # out col j=2f+s gets W_s[:,f] + W_{s+2}[:,f-1]
                res = rp.tile([128, 2 * FC], FP32)
                ov = res[:].rearrange("p (f two) -> p two f", two=2)
                for s01 in range(2):
                    pout = pwp.tile([128, FC], FP32)
                    for k in range(4):
                        nc.tensor.matmul(
                            pout[:],
                            lhsT=gsb[:, k, s01, :],
                            rhs=u_all[d * 4 + k][:, 1 + f0:1 + f0 + FC],
                            start=(k == 0),
                            stop=False,
                        )
                    for k in range(4):
                        nc.tensor.matmul(
                            pout[:],
                            lhsT=gsb[:, k, s01 + 2, :],
                            rhs=u_all[d * 4 + k][:, f0:f0 + FC],
                            start=False,
                            stop=(k == 3),
                        )
                    nc.scalar.activation(
                        ov[:, s01, :], pout[:],
                        mybir.ActivationFunctionType.Tanh,
                        scale=gain_sb[:, d:d + 1],
                    )
                oeng = {"scalar": nc.scalar, "sync": nc.sync,
                        "gpsimd": nc.gpsimd, "vector": nc.vector}[out_eng]
                oeng.dma_start(
                    out=out_d[d][:, 2 * f0:2 * (f0 + FC)],
                    in_=res[:])

            for _rep in range(reps):
                bigx = bigx_for(_rep)
                for cci in range(nchunk):
                    f0 = cum[cci]
                    FC = chunks[cci]
                    emit_until(chunk_need(_rep, cci))
                    # prefetch goal for the NEXT chunk (possibly next rep)
                    if cci < nchunk - 1:
                        goal = chunk_need(_rep, cci + 1)
                    elif _rep + 1 < reps and not serialize:
                        goal = chunk_need(_rep + 1, 0)
                    else:
                        goal = -1
                    for d in range(CPC):
                        if goal >= state["ptr"] and d % ilv == ilv - 1:
                            r, i = tile_plan[state["ptr"]]
                            emit_a_tile(bigx_for(r), i)
                            state["ptr"] += 1
                        # ---- folds ----
                        bxa = bigx[:, d, :, f0:f0 + FC]      # [128, 2, FC]
                        bxb = bigx[:, d, :, f0 + 1:f0 + FC + 1]
                        bx0 = bigx[:, d, 0, f0:f0 + FC + 1]
                        bx1 = bigx[:, d, 1, f0:f0 + FC + 1]
                        eh01 = eop.tile([128, 2, FC], BF16, tag="eh01")
                        c1t = eop.tile([128, FC], BF16, tag="c1")
                        c2t = eop.tile([128, FC], BF16, tag="c2")
                        b0t = eop.tile([128, FC], BF16, tag="b0")
                        b1t = eop.tile([128, FC], BF16, tag="b1")
                        eh0, eh1 = eh01[:, 0, :], eh01[:, 1, :]
                        nc.vector.tensor_tensor(eh01[:], bxa, bxb, op=ADD)
                        c1_eng = nc.gpsimd if pool_folds >= 4 else nc.vector
                        c1_eng.tensor_tensor(c1t[:], eh0, eh1, op=ADD)
                        c2_eng = nc.gpsimd if pool_folds >= 3 else nc.vector
                        c2_eng.tensor_tensor(c2t[:], eh0, eh1, op=SUB)
                        fold_eng = nc.gpsimd if pool_folds >= 2 else nc.vector
                        fold_eng.tensor_tensor(b0t[:], bx0[:, 0:FC], bx0[:, 1:FC + 1], op=SUB)
                        fold_eng2 = nc.gpsimd if pool_folds >= 1 else nc.vector
                        fold_eng2.tensor_tensor(b1t[:], bx1[:, 0:FC], bx1[:, 1:FC + 1], op=SUB)
                        # ---- forward DFT (radix-4) + scan per dof chunk ----
                        plan = [
                            [(0, c1t)],
                            [(1, c2t)],
                            [(2, b0t), (3, b1t)],
                            [(4, b0t), (5, b1t)],
                        ]
                        for m in range(4):
                            ps = psp.tile([128, FC], FP32)
                            terms = plan[m]
                            for ti, (blk, src) in enumerate(terms):
                                nc.tensor.matmul(
                                    ps[:],
                                    lhsT=fsb[:, blk, :],
                                    rhs=src[:],
                                    start=(ti == 0),
                                    stop=(ti == len(terms) - 1),
                                )
                            idx = d * 4 + m
                            u = u_all[idx]
                            init = 0.0 if cci == 0 else u[:, f0:f0 + 1]
                            scan_eng = nc.gpsimd if m < scan_pool else nc.vector
                            scan_eng.tensor_tensor_scan(
                                u[:, 1 + f0:1 + f0 + FC], ps[:],
                                trsb[:, idx:idx + 1].broadcast_to((128, FC)),
                                init, op0=ADD, op1=MUL,
                            )
                        pending.append((f0, FC, d))
                        if len(pending) > lag:
                            emit_inverse(*pending.pop(0))
                # drain; split the final channels' inverse into half-range
                # pipelines so the tail (inv -> tanh -> out DMA) drains in
                # half-size steps instead of one long serial chain
                for j, args in enumerate(pending):
                    fz, FCz, dz = args
                    if _rep == reps - 1 and j >= len(pending) - tail_split:
                        emit_inverse(fz, FCz // 2, dz)
                        emit_inverse(fz + FCz // 2, FCz - FCz // 2, dz)
                    else:
                        emit_inverse(*args)
                pending.clear()
    nc.compile()
    return nc


def build_in_maps(x, transfer, mixer_matrix, gain):
    f_blocks, g_l, newperm = _build_dft_matrices()

    # transfer per dof (re/im parts share the same real coefficient),
    # permuted into the chunked dof order
    tr_plain = np.empty((C, NDOF), np.float32)
    tr_plain[:, :NCOEF] = transfer
    tr_plain[:, NCOEF:] = transfer[:, 1:256]
    tr_dof = np.ascontiguousarray(tr_plain[:, newperm])

    in_maps = []
    for core in range(NCORES):
        b, h = core // 2, core % 2
        d0 = h * CPC
        mixcols = mixer_matrix[:, d0:d0 + CPC]               # [32, 16]
        mix4 = np.zeros((128, 4 * CPC), np.float16)
        for q in range(4):
            mix4[32 * q:32 * (q + 1), CPC * q:CPC * (q + 1)] = mixcols
        trd = tr_dof[d0:d0 + CPC]                            # [16, 512]
        trt = np.ascontiguousarray(
            trd.reshape(CPC, 4, 128).transpose(2, 0, 1).reshape(128, CPC * 4))
        # xq[32*bq + c, G*128 + p] = x[c, G*512 + bq*128 + p]
        xqv = np.ascontiguousarray(
            x[b].reshape(C, T // 512, 4, 128).transpose(2, 0, 1, 3)
            .reshape(128, T // 4).astype(np.float16))
        in_maps.append({
            "xq": xqv,
            "mix4": mix4,
            "fmat": f_blocks,
            "gmat": g_l,
            "trt": trt,
            "gainv": np.ascontiguousarray(gain[d0:d0 + CPC].reshape(1, CPC)),
        })
    return in_maps


_PROGRAM_CACHE = {}


def kernel(x, transfer, mixer_matrix, gain, **run_kwargs):
    x = np.ascontiguousarray(x, np.float32)
    transfer = np.asarray(transfer, np.float32)
    mixer_matrix = np.asarray(mixer_matrix, np.float32)
    gain = np.asarray(gain, np.float32)

    in_maps = build_in_maps(x, transfer, mixer_matrix, gain)

    if "nc" not in _PROGRAM_CACHE:
        _PROGRAM_CACHE["nc"] = _build_program()
    nc = _PROGRAM_CACHE["nc"]

    res = run_bass_kernel_spmd(nc, in_maps, list(range(NCORES)), **run_kwargs)

    out = np.empty((B, C, T), np.float32)
    for core in range(NCORES):
        b, h = core // 2, core % 2
        o = res.results[core]["out"]                    # [16, 128, 1024]
        out[b, h * CPC:(h + 1) * CPC] = o.transpose(0, 2, 1).reshape(CPC, T)
    kernel.last_results = res
    return out


# revision 32
# speedup vs baseline: 1.0742x; 1.0742x over previous
"""Trainium2 Bass kernel for nn_Block_9345848836513.

Per-core pipeline (8 cores = 4 batches x 2 channel-halves, 16 ch each):
  1. channel mix in fp16 on PE: lhsT = x-chunk [128 rows = 4 consecutive
     128-tau windows x 32 ch, 128 taus], rhs = block-diag 4x mixer
     [128, 64] -> psum [tau, (window, ch)]; full 128-row contraction,
     one psum tag.  x arrives from HBM as fp16 (halves the phase-A DMA,
     its critical path).  Copies scatter psum into BigX laid out
     [p, ch, j-parity, frame] (fp16) so later folds read contiguous rows.
  2. forward rfft of 512-sample frames (hop 256) via radix-4-folded real
     DFT: DVE/Pool folds produce C1/C2 (even-bin sources) and B0/B1
     (odd-bin halves); 6 bf16 matmuls per channel give all 512 real DOFs.
  3. per-frame recurrence out_i = (spec_i + out_{i-1}) * transfer via
     tensor_tensor_scan along the frame axis (fp32 state, bf16 out)
  4. inverse rfft with Hann folded into the bf16 matrix; overlap-add
     folded into PSUM accumulation (second matmul group reads with a
     one-column shift); tanh straight from PSUM.

Single-shot latency optimization: the whole transform chain is CHUNKED
along the frame axis (nchunk chunks of F/nchunk frames).  The scan
chains across chunks via its `initial` operand (the previous chunk's
last output column), so phase B for chunk c starts as soon as the x
tiles covering its frames (+1 boundary slot) have arrived - phase A DMA
and mix overlap phase B of earlier chunks instead of serializing in
front of the whole transform.  b0/b1 folds ride the Pool engine and
output DMAs ride the idle SP HWDGE queue to keep DVE under the PE
roofline.  u spectra live in 64 full-span [128,513] tiles so chunk
boundaries need no copies.
"""

import numpy as np

import concourse.bass as bass
import concourse.mybir as mybir
import concourse.tile as tile
from concourse import bacc
from concourse.bass_utils import run_bass_kernel_spmd

WINDOW = 512
HOP = 256
NCOEF = 257
NDOF = 512
B, C, T = 4, 32, 131072
F = T // HOP          # 512 frames
CPC = 16              # channels per core
NCORES = 8
JCOLS = T // 128      # 1024 output columns per channel
FPAD = F + 1          # 513 frame slots per (ch, parity); last is zero pad
FP32 = mybir.dt.float32
FP32R = mybir.dt.float32r
FP16 = mybir.dt.float16
BF16 = mybir.dt.bfloat16
U16 = mybir.dt.uint16


def _build_dft_matrices():
    w = np.arange(WINDOW)
    k = np.arange(NCOEF)
    ang = 2.0 * np.pi * np.outer(w, k) / WINDOW
    cos, sin = np.cos(ang), np.sin(ang)
    fmat = np.zeros((WINDOW, NDOF), np.float64)
    fmat[:, :NCOEF] = cos
    fmat[:, NCOEF:] = -sin[:, 1:256]
    hann = 0.5 - 0.5 * np.cos(2.0 * np.pi * w / WINDOW)
    g = np.zeros((NDOF, WINDOW), np.float64)
    g[0, :] = 1.0
    g[256, :] = cos[:, 256]
    for kk in range(1, 256):
        g[kk, :] = 2.0 * cos[:, kk]
        g[256 + kk, :] = -2.0 * sin[:, kk]
    g *= hann[None, :] / WINDOW

    # dof indexing in the plain layout: Re k -> k (0..256), Im k -> 256+k
    def dofs_re(ks):
        return list(ks)

    def dofs_im(ks):
        return [256 + kk for kk in ks if 1 <= kk <= 255]

    # Chunks of 128 dofs in bin-class order (radix-4): classes k mod 4 =
    # 0/2/1/3 with fold sources C1 = x0+x1+x2+x3, C2 = x0-x1+x2-x3 (via
    # e-halves eh0 = x0+x2, eh1 = x1+x3), B0 = x0-x2, B1 = x1-x3; for
    # k=0 mod 4: X[k] = sum_n C1[n] W^nk; k=2 mod 4: C2; odd k:
    # X[k] = sum_n B0[n] W^nk + B1[n] W^(n+128)k.
    chunk_k = [
        list(range(0, 257, 4)),
        list(range(2, 256, 4)),
        list(range(1, 256, 4)),
        list(range(3, 256, 4)),
    ]
    newperm = []
    for ks in chunk_k:
        newperm += dofs_re(ks) + dofs_im(ks)
    newperm = np.asarray(newperm)
    assert newperm.size == NDOF and np.unique(newperm).size == NDOF

    p0, p1, p2, p3 = (newperm[128 * i:128 * (i + 1)] for i in range(4))
    # 6 lhsT blocks: (C1->c0, C2->c1, B0->c2, B1->c2, B0->c3, B1->c3)
    f_blocks = np.stack([
        fmat[0:128, p0],
        fmat[0:128, p1],
        fmat[0:128, p2],
        fmat[128:256, p2],
        fmat[0:128, p3],
        fmat[128:256, p3],
    ], axis=1)                                    # [128, 6, 128]
    g2 = g[newperm, :]                                # [512, 512]
    g_l = g2.reshape(4, 128, 4, 128).transpose(1, 0, 2, 3)
    bf16 = mybir.dt.np(BF16)
    return (np.ascontiguousarray(f_blocks.astype(bf16)),
            np.ascontiguousarray(g_l.astype(bf16)), newperm)


def _build_program(detect_races=True, reps=1, chunks=(192, 320), pool_folds=2,
                   out_eng="scalar", psp_bufs=3, pwp_bufs=3, xa_bufs=3,
                   eo_bufs=3, rp_bufs=3, ilv=3, scan_pool=0, lag=1,
                   copy_split=("scalar", "scalar", "scalar", "scalar"),
                   serialize=False, tail_split=2, sched=None):
    nc = bacc.Bacc("TRN2", target_bir_lowering=False, num_devices=NCORES,
                   detect_race_conditions=detect_races)
    # xq[32*b + c, G*128 + p] = x[c, G*512 + b*128 + p]  (fp16, host layout)
    xq = nc.dram_tensor("xq", [128, T // 4], FP16, kind="ExternalInput")
    # mix4[32*b + c, 16*b + d] = mixer[c, d0+d]; zero elsewhere (fp16)
    mix4 = nc.dram_tensor("mix4", [128, 4 * CPC], FP16, kind="ExternalInput")
    fmat = nc.dram_tensor("fmat", [128, 6, 128], BF16, kind="ExternalInput")
    gmat = nc.dram_tensor("gmat", [128, 4, 4, 128], BF16, kind="ExternalInput")
    trt = nc.dram_tensor("trt", [128, CPC * 4], FP32, kind="ExternalInput")
    gainv = nc.dram_tensor("gainv", [1, CPC], FP32, kind="ExternalInput")
    out_d = nc.dram_tensor("out", [CPC, 128, JCOLS], FP32, kind="ExternalOutput")

    ADD, MUL = mybir.AluOpType.add, mybir.AluOpType.mult
    SUB = mybir.AluOpType.subtract

    chunks = tuple(chunks)
    nchunk = len(chunks)
    assert sum(chunks) == F and all(c > 0 for c in chunks)
    cum = [0]
    for c in chunks:
        cum.append(cum[-1] + c)

    with tile.TileContext(nc) as tc:
        with (
            tc.tile_pool(name="singles", bufs=1) as singles,
            tc.tile_pool(name="xa", bufs=xa_bufs) as xa,
            tc.tile_pool(name="pmix", bufs=2, space="PSUM") as pmix,
            tc.tile_pool(name="eo", bufs=eo_bufs) as eop,
            tc.tile_pool(name="rp", bufs=rp_bufs) as rp,
            tc.tile_pool(name="psp", bufs=psp_bufs, space="PSUM") as psp,
            tc.tile_pool(name="pwp", bufs=pwp_bufs, space="PSUM") as pwp,
        ):
            fsb = singles.tile([128, 6, 128], BF16)
            gsb = singles.tile([128, 4, 4, 128], BF16)
            mix_sb = singles.tile([128, 4 * CPC], FP16)
            trsb = singles.tile([128, CPC * 4], FP32)
            gain_sb = singles.tile([128, CPC], FP32)
            # bigx[p, d, h, f] = y[d, t = 256f + 128h + p]  (j = 2f+h)
            bigxs = [singles.tile([128, CPC, 2, FPAD], FP16, name=f"bigx{r}")
                     for r in range(min(reps, 2))]
            # one full-span u tile per (channel, dof-chunk): the scan writes
            # chunk frames at col offset 1, so col f0 is the previous
            # chunk's last output (scan initial + shifted OLA read) and
            # col 0 is the zero pad for frame -1.
            u_all = [singles.tile([128, FPAD], BF16, name=f"uall{i}")
                     for i in range(CPC * 4)]
            for ut in u_all:
                nc.vector.memset(ut[:, 0:1].bitcast(U16), 0)
            # preload the tanh activation table during the DMA-idle fill
            # window (the implicit load costs 1.28us and would otherwise
            # block phase-A psum copies at the first real tanh)
            warm = singles.tile([128, 1], FP32)
            nc.vector.memset(warm[:], 0)
            nc.scalar.activation(warm[:], warm[:],
                                 mybir.ActivationFunctionType.Tanh)
            # params ride the SWDGE queue so the first phase-A x-load
            # (HWDGE) isn't queued behind the parameter DMAs
            nc.sync.dma_start(out=mix_sb[:], in_=mix4[:])
            nc.gpsimd.dma_start(out=fsb[:], in_=fmat[:])
            nc.gpsimd.dma_start(out=gsb[:], in_=gmat[:])
            nc.gpsimd.dma_start(out=trsb[:], in_=trt[:])
            nc.gpsimd.dma_start(out=gain_sb[:], in_=gainv[:].to_broadcast((128, CPC)))
            for bx in bigxs:
                nc.vector.memset(bx[:, :, :, F:FPAD].bitcast(U16), 0)

            def bigx_copy(eng, dst, src):
                if eng == "scalar":
                    nc.scalar.copy(dst, src)
                elif eng == "vector":
                    nc.vector.tensor_scalar(dst, src, 0.0, None, op0=ADD)
                else:
                    nc.gpsimd.tensor_scalar(dst, src, 0.0, None, op0=ADD)

            def emit_a_tile(bigx, i):
                # ---- Phase A: transposed mix into BigX (fp16) ----
                # xt tile i holds groups G in [32i, 32(i+1)); matmul for
                # group G: lhsT = xt[:, local window] -> psum cols
                # [64*G' + 16*b + d] with j = 4G + b, h = b%2,
                # f = 2*(8s + G') + b//2  (s = psum slab index 4i+g).
                xt = xa.tile([128, 4096], FP16, tag="xt")
                nc.sync.dma_start(
                    out=xt[:], in_=xq[:, 4096 * i:4096 * (i + 1)])
                for g in range(4):
                    ps = pmix.tile([128, 512], FP32, tag="pmix")
                    for gp in range(8):
                        nc.tensor.matmul(
                            ps[:, 64 * gp:64 * (gp + 1)],
                            lhsT=xt[:, 1024 * g + 128 * gp:
                                    1024 * g + 128 * (gp + 1)],
                            rhs=mix_sb[:],
                        )
                    # psum col = G'*64 + b*16 + d, with b = 2*bf + h
                    # and frame offset fr = 2*G' + bf, i.e.
                    # col = 32*fr + 16*h + d -> "(fr h d)" grouping.
                    f0 = 16 * (4 * i + g)
                    bigx_copy(
                        copy_split[g],
                        bigx[:, :, :, f0:f0 + 16],
                        ps[:].rearrange("p (fr h d) -> p d h fr",
                                        fr=16, h=2),
                    )

            # global i-tile plan across reps; tiles are emitted lazily so
            # phase A interleaves with phase B of earlier chunks
            tile_plan = [(r, i) for r in range(reps) for i in range(8)]
            state = {"ptr": 0}

            def bigx_for(r):
                return bigxs[r % len(bigxs)]

            def emit_until(goal):
                # emit pending i-tiles up to global index `goal` (inclusive)
                while state["ptr"] <= min(goal, len(tile_plan) - 1):
                    r, i = tile_plan[state["ptr"]]
                    emit_a_tile(bigx_for(r), i)
                    state["ptr"] += 1

            pending = []

            def emit_inverse(f0, FC, d):
                # ---- inverse DFT with overlap-add in PSUM ----
                # out col j=2f+s gets W_s[:,f] + W_{s+2}[:,f-1]
                res = rp.tile([128, 2 * FC], FP32)
                ov = res[:].rearrange("p (f two) -> p two f", two=2)
                for s01 in range(2):
                    pout = pwp.tile([128, FC], FP32)
                    for k in range(4):
                        nc.tensor.matmul(
                            pout[:],
                            lhsT=gsb[:, k, s01, :],
                            rhs=u_all[d * 4 + k][:, 1 + f0:1 + f0 + FC],
                            start=(k == 0),
                            stop=False,
                        )
                    for k in range(4):
                        nc.tensor.matmul(
                            pout[:],
                            lhsT=gsb[:, k, s01 + 2, :],
                            rhs=u_all[d * 4 + k][:, f0:f0 + FC],
                            start=False,
                            stop=(k == 3),
                        )
                    nc.scalar.activation(
                        ov[:, s01, :], pout[:],
                        mybir.ActivationFunctionType.Tanh,
                        scale=gain_sb[:, d:d + 1],
                    )
                oeng = {"scalar": nc.scalar, "sync": nc.sync,
                        "gpsimd": nc.gpsimd, "vector": nc.vector}[out_eng]
                oeng.dma_start(
                    out=out_d[d][:, 2 * f0:2 * (f0 + FC)],
                    in_=res[:])

            # schedule: list of (f0, FC, d_lo, d_hi) entries processed in
            # order; default = each chunk over all channels.  A hybrid
            # schedule can run early channels chunked (starting during the
            # x DMA fill) and late channels in one full-span pass.
            if sched is None:
                sched = [(cum[c], chunks[c], 0, CPC) for c in range(nchunk)]
            covered = [0] * CPC
            for (sf0, sFC, dlo, dhi) in sched:
                for d in range(dlo, dhi):
                    assert covered[d] == sf0, (d, covered[d], sf0)
                    covered[d] += sFC
            assert all(c == F for c in covered)

            def entry_need(r, ent):
                # last global tile index the entry reads: through the tile
                # whose copy covers the fold boundary frame slot (the
                # zero pad at F needs no tile).
                f0, FC, dlo, dhi = ent
                return 8 * r + min((f0 + FC) // 64, 7)

            for _rep in range(reps):
                bigx = bigx_for(_rep)
                for eidx, ent in enumerate(sched):
                    f0, FC, dlo, dhi = ent
                    emit_until(entry_need(_rep, ent))
                    # prefetch goal for the NEXT entry (possibly next rep)
                    if eidx < len(sched) - 1:
                        goal = entry_need(_rep, sched[eidx + 1])
                    elif _rep + 1 < reps and not serialize:
                        goal = entry_need(_rep + 1, sched[0])
                    else:
                        goal = -1
                    for d in range(dlo, dhi):
                        if goal >= state["ptr"] and d % ilv == ilv - 1:
                            r, i = tile_plan[state["ptr"]]
                            emit_a_tile(bigx_for(r), i)
                            state["ptr"] += 1
                        # ---- folds ----
                        bxa = bigx[:, d, :, f0:f0 + FC]      # [128, 2, FC]
                        bxb = bigx[:, d, :, f0 + 1:f0 + FC + 1]
                        bx0 = bigx[:, d, 0, f0:f0 + FC + 1]
                        bx1 = bigx[:, d, 1, f0:f0 + FC + 1]
                        eh01 = eop.tile([128, 2, FC], BF16, tag="eh01")
                        c1t = eop.tile([128, FC], BF16, tag="c1")
                        c2t = eop.tile([128, FC], BF16, tag="c2")
                        b0t = eop.tile([128, FC], BF16, tag="b0")
                        b1t = eop.tile([128, FC], BF16, tag="b1")
                        eh0, eh1 = eh01[:, 0, :], eh01[:, 1, :]
                        nc.vector.tensor_tensor(eh01[:], bxa, bxb, op=ADD)
                        c1_eng = nc.gpsimd if pool_folds >= 4 else nc.vector
                        c1_eng.tensor_tensor(c1t[:], eh0, eh1, op=ADD)
                        c2_eng = nc.gpsimd if pool_folds >= 3 else nc.vector
                        c2_eng.tensor_tensor(c2t[:], eh0, eh1, op=SUB)
                        fold_eng = nc.gpsimd if pool_folds >= 2 else nc.vector
                        fold_eng.tensor_tensor(b0t[:], bx0[:, 0:FC], bx0[:, 1:FC + 1], op=SUB)
                        fold_eng2 = nc.gpsimd if pool_folds >= 1 else nc.vector
                        fold_eng2.tensor_tensor(b1t[:], bx1[:, 0:FC], bx1[:, 1:FC + 1], op=SUB)
                        # ---- forward DFT (radix-4) + scan per dof chunk ----
                        plan = [
                            [(0, c1t)],
                            [(1, c2t)],
                            [(2, b0t), (3, b1t)],
                            [(4, b0t), (5, b1t)],
                        ]
                        for m in range(4):
                            ps = psp.tile([128, FC], FP32)
                            terms = plan[m]
                            for ti, (blk, src) in enumerate(terms):
                                nc.tensor.matmul(
                                    ps[:],
                                    lhsT=fsb[:, blk, :],
                                    rhs=src[:],
                                    start=(ti == 0),
                                    stop=(ti == len(terms) - 1),
                                )
                            idx = d * 4 + m
                            u = u_all[idx]
                            init = 0.0 if f0 == 0 else u[:, f0:f0 + 1]
                            scan_eng = nc.gpsimd if m < scan_pool else nc.vector
                            scan_eng.tensor_tensor_scan(
                                u[:, 1 + f0:1 + f0 + FC], ps[:],
                                trsb[:, idx:idx + 1].broadcast_to((128, FC)),
                                init, op0=ADD, op1=MUL,
                            )
                        pending.append((f0, FC, d))
                        if len(pending) > lag:
                            emit_inverse(*pending.pop(0))
                # drain; split the final channels' inverse into half-range
                # pipelines so the tail (inv -> tanh -> out DMA) drains in
                # half-size steps instead of one long serial chain
                for j, args in enumerate(pending):
                    fz, FCz, dz = args
                    if _rep == reps - 1 and j >= len(pending) - tail_split:
                        emit_inverse(fz, FCz // 2, dz)
                        emit_inverse(fz + FCz // 2, FCz - FCz // 2, dz)
                    else:
                        emit_inverse(*args)
                pending.clear()
    nc.compile()
    return nc


def build_in_maps(x, transfer, mixer_matrix, gain):
    f_blocks, g_l, newperm = _build_dft_matrices()

    # transfer per dof (re/im parts share the same real coefficient),
    # permuted into the chunked dof order
    tr_plain = np.empty((C, NDOF), np.float32)
    tr_plain[:, :NCOEF] = transfer
    tr_plain[:, NCOEF:] = transfer[:, 1:256]
    tr_dof = np.ascontiguousarray(tr_plain[:, newperm])

    in_maps = []
    for core in range(NCORES):
        b, h = core // 2, core % 2
        d0 = h * CPC
        mixcols = mixer_matrix[:, d0:d0 + CPC]               # [32, 16]
        mix4 = np.zeros((128, 4 * CPC), np.float16)
        for q in range(4):
            mix4[32 * q:32 * (q + 1), CPC * q:CPC * (q + 1)] = mixcols
        trd = tr_dof[d0:d0 + CPC]                            # [16, 512]
        trt = np.ascontiguousarray(
            trd.reshape(CPC, 4, 128).transpose(2, 0, 1).reshape(128, CPC * 4))
        # xq[32*bq + c, G*128 + p] = x[c, G*512 + bq*128 + p]
        xqv = np.ascontiguousarray(
            x[b].reshape(C, T // 512, 4, 128).transpose(2, 0, 1, 3)
            .reshape(128, T // 4).astype(np.float16))
        in_maps.append({
            "xq": xqv,
            "mix4": mix4,
            "fmat": f_blocks,
            "gmat": g_l,
            "trt": trt,
            "gainv": np.ascontiguousarray(gain[d0:d0 + CPC].reshape(1, CPC)),
        })
    return in_maps


_PROGRAM_CACHE = {}


def kernel(x, transfer, mixer_matrix, gain, **run_kwargs):
    x = np.ascontiguousarray(x, np.float32)
    transfer = np.asarray(transfer, np.float32)
    mixer_matrix = np.asarray(mixer_matrix, np.float32)
    gain = np.asarray(gain, np.float32)

    in_maps = build_in_maps(x, transfer, mixer_matrix, gain)

    if "nc" not in _PROGRAM_CACHE:
        _PROGRAM_CACHE["nc"] = _build_program()
    nc = _PROGRAM_CACHE["nc"]

    res = run_bass_kernel_spmd(nc, in_maps, list(range(NCORES)), **run_kwargs)

    out = np.empty((B, C, T), np.float32)
    for core in range(NCORES):
        b, h = core // 2, core % 2
        o = res.results[core]["out"]                    # [16, 128, 1024]
        out[b, h * CPC:(h + 1) * CPC] = o.transpose(0, 2, 1).reshape(CPC, T)
    kernel.last_results = res
    return out


# revision 34
# speedup vs baseline: 1.4554x; 1.3548x over previous
"""Trainium2 Bass kernel for nn_Block_9345848836513.

Per-core pipeline (8 cores = 4 batches x 2 channel-halves, 16 ch each):
  1. channel mix in fp16 on PE: lhsT = x-chunk [128 rows = 4 consecutive
     128-tau windows x 32 ch, 128 taus], rhs = block-diag 4x mixer
     [128, 64] -> psum [tau, (window, ch)]; full 128-row contraction,
     one psum tag.  x arrives from HBM as fp16 (halves the phase-A DMA,
     its critical path).  Copies scatter psum into BigX laid out
     [p, ch, j-parity, frame] (fp16) so later folds read contiguous rows.
  2. forward rfft of 512-sample frames (hop 256) via radix-4-folded real
     DFT: DVE/Pool folds produce C1/C2 (even-bin sources) and B0/B1
     (odd-bin halves); 6 bf16 matmuls per channel give all 512 real DOFs.
  3. per-frame recurrence out_i = (spec_i + out_{i-1}) * transfer via
     tensor_tensor_scan along the frame axis (fp32 state, bf16 out)
  4. inverse rfft with Hann folded into the bf16 matrix; overlap-add
     folded into PSUM accumulation (second matmul group reads with a
     one-column shift); tanh straight from PSUM.

Single-shot latency optimization: the whole transform chain is CHUNKED
along the frame axis (default chunks 192+320).  The scan chains across
chunks via its `initial` operand (the previous chunk's last output
column, bf16 carry), so phase B for chunk c starts as soon as the x
tiles covering its frames (+1 boundary slot) have arrived - phase A DMA
and mix overlap phase B of earlier chunks instead of serializing in
front of the whole transform (the fill was ~30us of the old 118us sim
single-shot; now ~12us).  Engine balance: b0/b1 folds ride Pool (GPSIMD
cannot touch PSUM, so psum-reading ops stay off it), phase-A psum->BigX
copies and the tanh ride Activation, output DMAs ride the Act HWDGE
queue (keeping the SP queue exclusively for x loads - an out-DMA queued
ahead of an x load would stall the fill on compute), and scans stay on
DVE.  `lag=1` software-pipelines each channel's inverse behind the next
channel's forward so PE never waits on a scan.  The tanh act table is
preloaded during the fill (the implicit load is 1.3us).  The final two
channels' inverses split into half-range pipelines to shorten the
drain.  u spectra live in 64 full-span [128,513] bf16 tiles so chunk
boundaries need no copies.  Sim single-shot: 104.0us vs 117.7us for the
previous unchunked kernel (PE busy 83.5us is the roofline; the inverse
OLA's 2x512-dof contraction per output sample is irreducible without
fp8, which the 2e-2 gate does not permit).
"""

import numpy as np

import concourse.bass as bass
import concourse.mybir as mybir
import concourse.tile as tile
from concourse import bacc
from concourse.bass_utils import run_bass_kernel_spmd

WINDOW = 512
HOP = 256
NCOEF = 257
NDOF = 512
B, C, T = 4, 32, 131072
F = T // HOP          # 512 frames
CPC = 16              # channels per core
NCORES = 8
JCOLS = T // 128      # 1024 output columns per channel
FPAD = F + 1          # 513 frame slots per (ch, parity); last is zero pad
FP32 = mybir.dt.float32
FP32R = mybir.dt.float32r
FP16 = mybir.dt.float16
BF16 = mybir.dt.bfloat16
U16 = mybir.dt.uint16


def _build_dft_matrices():
    w = np.arange(WINDOW)
    k = np.arange(NCOEF)
    ang = 2.0 * np.pi * np.outer(w, k) / WINDOW
    cos, sin = np.cos(ang), np.sin(ang)
    fmat = np.zeros((WINDOW, NDOF), np.float64)
    fmat[:, :NCOEF] = cos
    fmat[:, NCOEF:] = -sin[:, 1:256]
    hann = 0.5 - 0.5 * np.cos(2.0 * np.pi * w / WINDOW)
    g = np.zeros((NDOF, WINDOW), np.float64)
    g[0, :] = 1.0
    g[256, :] = cos[:, 256]
    for kk in range(1, 256):
        g[kk, :] = 2.0 * cos[:, kk]
        g[256 + kk, :] = -2.0 * sin[:, kk]
    g *= hann[None, :] / WINDOW

    # dof indexing in the plain layout: Re k -> k (0..256), Im k -> 256+k
    def dofs_re(ks):
        return list(ks)

    def dofs_im(ks):
        return [256 + kk for kk in ks if 1 <= kk <= 255]

    # Chunks of 128 dofs in bin-class order (radix-4): classes k mod 4 =
    # 0/2/1/3 with fold sources C1 = x0+x1+x2+x3, C2 = x0-x1+x2-x3 (via
    # e-halves eh0 = x0+x2, eh1 = x1+x3), B0 = x0-x2, B1 = x1-x3; for
    # k=0 mod 4: X[k] = sum_n C1[n] W^nk; k=2 mod 4: C2; odd k:
    # X[k] = sum_n B0[n] W^nk + B1[n] W^(n+128)k.
    chunk_k = [
        list(range(0, 257, 4)),
        list(range(2, 256, 4)),
        list(range(1, 256, 4)),
        list(range(3, 256, 4)),
    ]
    newperm = []
    for ks in chunk_k:
        newperm += dofs_re(ks) + dofs_im(ks)
    newperm = np.asarray(newperm)
    assert newperm.size == NDOF and np.unique(newperm).size == NDOF

    p0, p1, p2, p3 = (newperm[128 * i:128 * (i + 1)] for i in range(4))
    # 6 lhsT blocks: (C1->c0, C2->c1, B0->c2, B1->c2, B0->c3, B1->c3)
    f_blocks = np.stack([
        fmat[0:128, p0],
        fmat[0:128, p1],
        fmat[0:128, p2],
        fmat[128:256, p2],
        fmat[0:128, p3],
        fmat[128:256, p3],
    ], axis=1)                                    # [128, 6, 128]
    g2 = g[newperm, :]                                # [512, 512]
    g_l = g2.reshape(4, 128, 4, 128).transpose(1, 0, 2, 3)
    bf16 = mybir.dt.np(BF16)
    return (np.ascontiguousarray(f_blocks.astype(bf16)),
            np.ascontiguousarray(g_l.astype(bf16)), newperm)


def _build_program(detect_races=True, reps=1, chunks=(192, 320), pool_folds=2,
                   out_eng="scalar", psp_bufs=3, pwp_bufs=3, xa_bufs=3,
                   eo_bufs=3, rp_bufs=3, ilv=3, scan_pool=0, lag=1,
                   copy_split=("scalar", "scalar", "scalar", "scalar"),
                   serialize=False, tail_split=2, sched=None):
    nc = bacc.Bacc("TRN2", target_bir_lowering=False, num_devices=NCORES,
                   detect_race_conditions=detect_races)
    # xq[32*b + c, G*128 + p] = x[c, G*512 + b*128 + p]  (fp16, host layout)
    xq = nc.dram_tensor("xq", [128, T // 4], FP16, kind="ExternalInput")
    # mix4[32*b + c, 16*b + d] = mixer[c, d0+d]; zero elsewhere (fp16)
    mix4 = nc.dram_tensor("mix4", [128, 4 * CPC], FP16, kind="ExternalInput")
    fmat = nc.dram_tensor("fmat", [128, 6, 128], BF16, kind="ExternalInput")
    gmat = nc.dram_tensor("gmat", [128, 4, 4, 128], BF16, kind="ExternalInput")
    trt = nc.dram_tensor("trt", [128, CPC * 4], FP32, kind="ExternalInput")
    gainv = nc.dram_tensor("gainv", [1, CPC], FP32, kind="ExternalInput")
    out_d = nc.dram_tensor("out", [CPC, 128, JCOLS], FP32, kind="ExternalOutput")

    ADD, MUL = mybir.AluOpType.add, mybir.AluOpType.mult
    SUB = mybir.AluOpType.subtract

    chunks = tuple(chunks)
    nchunk = len(chunks)
    assert sum(chunks) == F and all(c > 0 for c in chunks)
    # GPSIMD cannot access PSUM on TRN2 (BIR verifier rejects it); the
    # scan reads the forward-DFT psum, so it must stay on DVE.
    assert scan_pool == 0
    assert "gpsimd" not in copy_split
    cum = [0]
    for c in chunks:
        cum.append(cum[-1] + c)

    with tile.TileContext(nc) as tc:
        with (
            tc.tile_pool(name="singles", bufs=1) as singles,
            tc.tile_pool(name="xa", bufs=xa_bufs) as xa,
            tc.tile_pool(name="pmix", bufs=2, space="PSUM") as pmix,
            tc.tile_pool(name="eo", bufs=eo_bufs) as eop,
            tc.tile_pool(name="rp", bufs=rp_bufs) as rp,
            tc.tile_pool(name="psp", bufs=psp_bufs, space="PSUM") as psp,
            tc.tile_pool(name="pwp", bufs=pwp_bufs, space="PSUM") as pwp,
        ):
            fsb = singles.tile([128, 6, 128], BF16)
            gsb = singles.tile([128, 4, 4, 128], BF16)
            mix_sb = singles.tile([128, 4 * CPC], FP16)
            trsb = singles.tile([128, CPC * 4], FP32)
            gain_sb = singles.tile([128, CPC], FP32)
            # bigx[p, d, h, f] = y[d, t = 256f + 128h + p]  (j = 2f+h)
            bigxs = [singles.tile([128, CPC, 2, FPAD], FP16, name=f"bigx{r}")
                     for r in range(min(reps, 2))]
            # one full-span u tile per (channel, dof-chunk): the scan writes
            # chunk frames at col offset 1, so col f0 is the previous
            # chunk's last output (scan initial + shifted OLA read) and
            # col 0 is the zero pad for frame -1.
            u_all = [singles.tile([128, FPAD], BF16, name=f"uall{i}")
                     for i in range(CPC * 4)]
            for ut in u_all:
                nc.vector.memset(ut[:, 0:1].bitcast(U16), 0)
            # preload the tanh activation table during the DMA-idle fill
            # window (the implicit load costs 1.28us and would otherwise
            # block phase-A psum copies at the first real tanh)
            warm = singles.tile([128, 1], FP32)
            nc.vector.memset(warm[:], 0)
            nc.scalar.activation(warm[:], warm[:],
                                 mybir.ActivationFunctionType.Tanh)
            # params ride the SWDGE queue so the first phase-A x-load
            # (HWDGE) isn't queued behind the parameter DMAs
            nc.sync.dma_start(out=mix_sb[:], in_=mix4[:])
            nc.gpsimd.dma_start(out=fsb[:], in_=fmat[:])
            nc.gpsimd.dma_start(out=gsb[:], in_=gmat[:])
            nc.gpsimd.dma_start(out=trsb[:], in_=trt[:])
            nc.gpsimd.dma_start(out=gain_sb[:], in_=gainv[:].to_broadcast((128, CPC)))
            for bx in bigxs:
                nc.vector.memset(bx[:, :, :, F:FPAD].bitcast(U16), 0)

            def bigx_copy(eng, dst, src):
                if eng == "scalar":
                    nc.scalar.copy(dst, src)
                elif eng == "vector":
                    nc.vector.tensor_scalar(dst, src, 0.0, None, op0=ADD)
                else:
                    nc.gpsimd.tensor_scalar(dst, src, 0.0, None, op0=ADD)

            def emit_a_tile(bigx, i):
                # ---- Phase A: transposed mix into BigX (fp16) ----
                # xt tile i holds groups G in [32i, 32(i+1)); matmul for
                # group G: lhsT = xt[:, local window] -> psum cols
                # [64*G' + 16*b + d] with j = 4G + b, h = b%2,
                # f = 2*(8s + G') + b//2  (s = psum slab index 4i+g).
                xt = xa.tile([128, 4096], FP16, tag="xt")
                nc.sync.dma_start(
                    out=xt[:], in_=xq[:, 4096 * i:4096 * (i + 1)])
                for g in range(4):
                    ps = pmix.tile([128, 512], FP32, tag="pmix")
                    for gp in range(8):
                        nc.tensor.matmul(
                            ps[:, 64 * gp:64 * (gp + 1)],
                            lhsT=xt[:, 1024 * g + 128 * gp:
                                    1024 * g + 128 * (gp + 1)],
                            rhs=mix_sb[:],
                        )
                    # psum col = G'*64 + b*16 + d, with b = 2*bf + h
                    # and frame offset fr = 2*G' + bf, i.e.
                    # col = 32*fr + 16*h + d -> "(fr h d)" grouping.
                    f0 = 16 * (4 * i + g)
                    bigx_copy(
                        copy_split[g],
                        bigx[:, :, :, f0:f0 + 16],
                        ps[:].rearrange("p (fr h d) -> p d h fr",
                                        fr=16, h=2),
                    )

            # global i-tile plan across reps; tiles are emitted lazily so
            # phase A interleaves with phase B of earlier chunks
            tile_plan = [(r, i) for r in range(reps) for i in range(8)]
            state = {"ptr": 0}

            def bigx_for(r):
                return bigxs[r % len(bigxs)]

            def emit_until(goal):
                # emit pending i-tiles up to global index `goal` (inclusive)
                while state["ptr"] <= min(goal, len(tile_plan) - 1):
                    r, i = tile_plan[state["ptr"]]
                    emit_a_tile(bigx_for(r), i)
                    state["ptr"] += 1

            pending = []

            def emit_inverse(f0, FC, d):
                # ---- inverse DFT with overlap-add in PSUM ----
                # out col j=2f+s gets W_s[:,f] + W_{s+2}[:,f-1]
                res = rp.tile([128, 2 * FC], FP32)
                ov = res[:].rearrange("p (f two) -> p two f", two=2)
                for s01 in range(2):
                    pout = pwp.tile([128, FC], FP32)
                    for k in range(4):
                        nc.tensor.matmul(
                            pout[:],
                            lhsT=gsb[:, k, s01, :],
                            rhs=u_all[d * 4 + k][:, 1 + f0:1 + f0 + FC],
                            start=(k == 0),
                            stop=False,
                        )
                    for k in range(4):
                        nc.tensor.matmul(
                            pout[:],
                            lhsT=gsb[:, k, s01 + 2, :],
                            rhs=u_all[d * 4 + k][:, f0:f0 + FC],
                            start=False,
                            stop=(k == 3),
                        )
                    nc.scalar.activation(
                        ov[:, s01, :], pout[:],
                        mybir.ActivationFunctionType.Tanh,
                        scale=gain_sb[:, d:d + 1],
                    )
                oeng = {"scalar": nc.scalar, "sync": nc.sync,
                        "gpsimd": nc.gpsimd, "vector": nc.vector}[out_eng]
                oeng.dma_start(
                    out=out_d[d][:, 2 * f0:2 * (f0 + FC)],
                    in_=res[:])

            # schedule: list of (f0, FC, d_lo, d_hi) entries processed in
            # order; default = each chunk over all channels.  A hybrid
            # schedule can run early channels chunked (starting during the
            # x DMA fill) and late channels in one full-span pass.
            if sched is None:
                sched = [(cum[c], chunks[c], 0, CPC) for c in range(nchunk)]
            covered = [0] * CPC
            for (sf0, sFC, dlo, dhi) in sched:
                for d in range(dlo, dhi):
                    assert covered[d] == sf0, (d, covered[d], sf0)
                    covered[d] += sFC
            assert all(c == F for c in covered)

            def entry_need(r, ent):
                # last global tile index the entry reads: through the tile
                # whose copy covers the fold boundary frame slot (the
                # zero pad at F needs no tile).
                f0, FC, dlo, dhi = ent
                return 8 * r + min((f0 + FC) // 64, 7)

            for _rep in range(reps):
                bigx = bigx_for(_rep)
                for eidx, ent in enumerate(sched):
                    f0, FC, dlo, dhi = ent
                    emit_until(entry_need(_rep, ent))
                    # prefetch goal for the NEXT entry (possibly next rep)
                    if eidx < len(sched) - 1:
                        goal = entry_need(_rep, sched[eidx + 1])
                    elif _rep + 1 < reps and not serialize:
                        goal = entry_need(_rep + 1, sched[0])
                    else:
                        goal = -1
                    for d in range(dlo, dhi):
                        if goal >= state["ptr"] and d % ilv == ilv - 1:
                            r, i = tile_plan[state["ptr"]]
                            emit_a_tile(bigx_for(r), i)
                            state["ptr"] += 1
                        # ---- folds ----
                        bxa = bigx[:, d, :, f0:f0 + FC]      # [128, 2, FC]
                        bxb = bigx[:, d, :, f0 + 1:f0 + FC + 1]
                        bx0 = bigx[:, d, 0, f0:f0 + FC + 1]
                        bx1 = bigx[:, d, 1, f0:f0 + FC + 1]
                        eh01 = eop.tile([128, 2, FC], BF16, tag="eh01")
                        c1t = eop.tile([128, FC], BF16, tag="c1")
                        c2t = eop.tile([128, FC], BF16, tag="c2")
                        b0t = eop.tile([128, FC], BF16, tag="b0")
                        b1t = eop.tile([128, FC], BF16, tag="b1")
                        eh0, eh1 = eh01[:, 0, :], eh01[:, 1, :]
                        nc.vector.tensor_tensor(eh01[:], bxa, bxb, op=ADD)
                        c1_eng = nc.gpsimd if pool_folds >= 4 else nc.vector
                        c1_eng.tensor_tensor(c1t[:], eh0, eh1, op=ADD)
                        c2_eng = nc.gpsimd if pool_folds >= 3 else nc.vector
                        c2_eng.tensor_tensor(c2t[:], eh0, eh1, op=SUB)
                        fold_eng = nc.gpsimd if pool_folds >= 2 else nc.vector
                        fold_eng.tensor_tensor(b0t[:], bx0[:, 0:FC], bx0[:, 1:FC + 1], op=SUB)
                        fold_eng2 = nc.gpsimd if pool_folds >= 1 else nc.vector
                        fold_eng2.tensor_tensor(b1t[:], bx1[:, 0:FC], bx1[:, 1:FC + 1], op=SUB)
                        # ---- forward DFT (radix-4) + scan per dof chunk ----
                        plan = [
                            [(0, c1t)],
                            [(1, c2t)],
                            [(2, b0t), (3, b1t)],
                            [(4, b0t), (5, b1t)],
                        ]
                        for m in range(4):
                            ps = psp.tile([128, FC], FP32)
                            terms = plan[m]
                            for ti, (blk, src) in enumerate(terms):
                                nc.tensor.matmul(
                                    ps[:],
                                    lhsT=fsb[:, blk, :],
                                    rhs=src[:],
                                    start=(ti == 0),
                                    stop=(ti == len(terms) - 1),
                                )
                            idx = d * 4 + m
                            u = u_all[idx]
                            init = 0.0 if f0 == 0 else u[:, f0:f0 + 1]
                            scan_eng = nc.gpsimd if m < scan_pool else nc.vector
                            scan_eng.tensor_tensor_scan(
                                u[:, 1 + f0:1 + f0 + FC], ps[:],
                                trsb[:, idx:idx + 1].broadcast_to((128, FC)),
                                init, op0=ADD, op1=MUL,
                            )
                        pending.append((f0, FC, d))
                        if len(pending) > lag:
                            emit_inverse(*pending.pop(0))
                # drain; split the final channels' inverse into half-range
                # pipelines so the tail (inv -> tanh -> out DMA) drains in
                # half-size steps instead of one long serial chain
                for j, args in enumerate(pending):
                    fz, FCz, dz = args
                    if _rep == reps - 1 and j >= len(pending) - tail_split:
                        emit_inverse(fz, FCz // 2, dz)
                        emit_inverse(fz + FCz // 2, FCz - FCz // 2, dz)
                    else:
                        emit_inverse(*args)
                pending.clear()
    nc.compile()
    return nc


def build_in_maps(x, transfer, mixer_matrix, gain):
    f_blocks, g_l, newperm = _build_dft_matrices()

    # transfer per dof (re/im parts share the same real coefficient),
    # permuted into the chunked dof order
    tr_plain = np.empty((C, NDOF), np.float32)
    tr_plain[:, :NCOEF] = transfer
    tr_plain[:, NCOEF:] = transfer[:, 1:256]
    tr_dof = np.ascontiguousarray(tr_plain[:, newperm])

    in_maps = []
    for core in range(NCORES):
        b, h = core // 2, core % 2
        d0 = h * CPC
        mixcols = mixer_matrix[:, d0:d0 + CPC]               # [32, 16]
        mix4 = np.zeros((128, 4 * CPC), np.float16)
        for q in range(4):
            mix4[32 * q:32 * (q + 1), CPC * q:CPC * (q + 1)] = mixcols
        trd = tr_dof[d0:d0 + CPC]                            # [16, 512]
        trt = np.ascontiguousarray(
            trd.reshape(CPC, 4, 128).transpose(2, 0, 1).reshape(128, CPC * 4))
        # xq[32*bq + c, G*128 + p] = x[c, G*512 + bq*128 + p]
        xqv = np.ascontiguousarray(
            x[b].reshape(C, T // 512, 4, 128).transpose(2, 0, 1, 3)
            .reshape(128, T // 4).astype(np.float16))
        in_maps.append({
            "xq": xqv,
            "mix4": mix4,
            "fmat": f_blocks,
            "gmat": g_l,
            "trt": trt,
            "gainv": np.ascontiguousarray(gain[d0:d0 + CPC].reshape(1, CPC)),
        })
    return in_maps


_PROGRAM_CACHE = {}


def kernel(x, transfer, mixer_matrix, gain, **run_kwargs):
    x = np.ascontiguousarray(x, np.float32)
    transfer = np.asarray(transfer, np.float32)
    mixer_matrix = np.asarray(mixer_matrix, np.float32)
    gain = np.asarray(gain, np.float32)

    in_maps = build_in_maps(x, transfer, mixer_matrix, gain)

    if "nc" not in _PROGRAM_CACHE:
        _PROGRAM_CACHE["nc"] = _build_program()
    nc = _PROGRAM_CACHE["nc"]

    res = run_bass_kernel_spmd(nc, in_maps, list(range(NCORES)), **run_kwargs)

    out = np.empty((B, C, T), np.float32)
    for core in range(NCORES):
        b, h = core // 2, core % 2
        o = res.results[core]["out"]                    # [16, 128, 1024]
        out[b, h * CPC:(h + 1) * CPC] = o.transpose(0, 2, 1).reshape(CPC, T)
    kernel.last_results = res
    return out
